# revision 12
# baseline (speedup 1.0000x reference)
"""CMRGCN Trainium2 kernel (v3).

Sharding: data-parallel over batch B=8 across the 8 NeuronCores (core b gets
batch b). Adjacency / neighbor weights / fused relation weights are replicated.

Key structure (N=500 padded to NP=512):
  g[mt]  [128n, cb(3), T, MIX, DIM] bf16  — node-major snapshots (x, h1, h2)
  adjacency MM: lhsT = g-slice [n, (i,d)] (stationary, shared across the two
      relation graphs -> deduped weight load), moving = adj [n, m] per graph,
      psum P^T [(i,d), (tg, m)] accumulated over 4 n-chunks.
  weight MM: lhsT = fused W block [128, 128], moving = ptg [(j,d), (t, m)].
  nonlinear terms (4 relu + 2 tanh per layer) are written as bf16 tmp tiles
      (DVE tensor_scalar / Scalar ACT) and tree-summed in bf16 straight into
      hb (no f32 h tensor, no sequential accumulation chain).
  hconst (the j==i tanh(bias) constant) rides a virtual node: row N of the
      padded adjacency / gather matrices holds per-column sums, and g row N
      (partition 116 of the last node tile) is DMA-loaded with the constant
      after each layer's transposes, so the constant propagates through the
      matmuls for free.
  gather MM: lhsT = g-slice [n, (i,d)], moving = wg [n, m] (host-densified
      neighbor weights), interleaved into layer pipeline slots to keep the PE
      busy; psum drained in one [128,1024] op per (block, t) into a single
      osb [128, NG, NP, T] staging tile, DMA'd out as contiguous
      [64, 500, 12] blocks.
  A custom pass drops back-to-back redundant LDWEIGHTS (same stationary AP)
      before compile — bass emits one per matmul otherwise.
"""

import os
import numpy as np

B, T, N, DIM = 8, 12, 500, 64
N_MIX, N_LAYERS, N_HEADS, N_REL, NG, K = 2, 2, 4, 8, 2, 20
NP = 512          # padded node count
KT = NP // 128    # node tiles
C = DIM * (N_LAYERS + 1)   # 192 channels per mix in g
NCORES = 8

_BUILT = {}


def _rel(tg, i, j):
    return (tg * N_MIX + i) * N_MIX + j


def _dedupe_ldweights(nc):
    """Drop Ldweights that reload the stationary operand already in the PE
    array (identical AP, no sync side effects)."""
    ndrop = 0
    for fn in nc.m.functions:
        for blk in fn.blocks:
            insts = blk.instructions
            cur_key = None
            keep = []
            for inst in insts:
                op = inst.opcode
                if op == "Ldweights":
                    key = (str(inst.ins[0]), str(inst.is_transpose),
                           str(inst.perf_mode), str(inst.tile_position))
                    si = inst.sync_info
                    ups = list(si.on_update) if si is not None else []
                    waits = list(si.on_wait) if si is not None else []
                    if key == cur_key and not ups and not waits:
                        ndrop += 1
                        continue
                    cur_key = key
                keep.append(inst)
            if len(keep) != len(insts):
                while len(blk.instructions):
                    blk.instructions.pop()
                for inst in keep:
                    blk.instructions.append(inst)
    return ndrop


def _build():
    """Build + trace the single-core SPMD Bass program once."""
    if "nc" in _BUILT:
        return _BUILT["nc"]

    from contextlib import ExitStack
    import concourse.bass as bass
    import concourse.tile as tile
    import concourse.mybir as mybir
    from concourse import bacc
    from concourse.masks import make_identity

    f32 = mybir.dt.float32
    bf16 = mybir.dt.bfloat16
    AF = mybir.ActivationFunctionType
    ALU = mybir.AluOpType

    nc = bacc.Bacc("TRN2", target_bir_lowering=False, debug=False)

    xn_d = nc.dram_tensor("xn", [NP, T, N_MIX, DIM], bf16, kind="ExternalInput").ap()
    adj_d = nc.dram_tensor("adjb", [KT, 128, NG, NP], bf16, kind="ExternalInput").ap()
    wg_d = nc.dram_tensor("wgb", [KT, 128, NG, NP], bf16, kind="ExternalInput").ap()
    wmm_d = nc.dram_tensor("wmm", [128, 12, 128], bf16, kind="ExternalInput").ap()
    bias_d = nc.dram_tensor("bias", [128, 16], f32, kind="ExternalInput").ap()
    # virtual-node rows: partitions 116..127 of the last node tile for
    # c-blocks 1..2 (row 0 = hconst, rest zero), loaded once at startup
    hc_d = nc.dram_tensor("hc", [12, N_LAYERS, T, N_MIX, DIM], bf16,
                          kind="ExternalInput").ap()
    out_d = [
        nc.dram_tensor(f"out{i}", [NG * C, N, T], f32, kind="ExternalOutput").ap()
        for i in range(N_MIX)
    ]

    with tile.TileContext(nc) as tc, ExitStack() as ctx:
        wpool = ctx.enter_context(tc.tile_pool(name="wpool", bufs=1))
        gpool = ctx.enter_context(tc.tile_pool(name="gpool", bufs=1))
        ampool = ctx.enter_context(tc.tile_pool(name="ampool", bufs=1))
        ptgpool = ctx.enter_context(tc.tile_pool(name="ptgpool", bufs=1))
        hpool = ctx.enter_context(tc.tile_pool(name="hpool", bufs=1))
        tmppool = ctx.enter_context(tc.tile_pool(name="tmppool", bufs=12))
        osbpool = ctx.enter_context(tc.tile_pool(name="osbpool", bufs=1))
        psA = ctx.enter_context(tc.tile_pool(name="psA", bufs=2, space="PSUM"))
        psW = ctx.enter_context(tc.tile_pool(name="psW", bufs=2, space="PSUM"))

        # --- constants / weights to SBUF ---
        wmm_sb = wpool.tile([128, 12, 128], bf16, name="wmm_sb")
        nc.sync.dma_start(out=wmm_sb[:], in_=wmm_d[:])
        bias_sb = wpool.tile([128, 16], f32, name="bias_sb")
        nc.sync.dma_start(out=bias_sb[:], in_=bias_d[:])
        ident = wpool.tile([128, 128], bf16, name="ident")
        make_identity(nc, ident[:])


        def c_wblk(l, tg, j):
            return wmm_sb[:, (l * NG + tg) * 2 + j, :]

        def d_wblk(l, tg):
            return wmm_sb[:, 8 + l * NG + tg, :]

        # bias columns per layer l: 0: bc(tg0,j0) for the ACT term,
        # 1..3: negated bc for the three STT relu terms, 4: their sum (Bstar),
        # 5..6: d-path biases. Layer stride 7.
        def cb_act(l):
            return bias_sb[:, 7 * l:7 * l + 1]

        def cb_neg(l, k):
            return bias_sb[:, 7 * l + 1 + k:7 * l + 2 + k]

        def cb_star(l):
            return bias_sb[:, 7 * l + 4:7 * l + 5]

        def d_bias(l, tg):
            return bias_sb[:, 7 * l + 5 + tg:7 * l + 6 + tg]

        # --- inputs: x into g c-block 0; adjacency + gather-weight tiles ---
        g = []
        for mt in range(KT):
            gt = gpool.tile([128, 3, T, N_MIX, DIM], bf16, name=f"g{mt}", tag=f"g{mt}")
            g.append(gt)
        for mt in range(KT):
            nc.sync.dma_start(
                out=g[mt][:, 0, :, :, :],
                in_=xn_d[mt * 128:(mt + 1) * 128, :, :, :],
            )

        # virtual-node rows (node N = partition 116 of the last node tile,
        # plus zeroed pad rows) for c-blocks 1..2: written once here; the
        # transpose drains below never touch partitions >= 116 of g[3]
        nc.sync.dma_start(out=g[KT - 1][116:128, 1:3, :, :, :], in_=hc_d[:])

        adj_sb, wg_sb = [], []
        for kt in range(KT):
            a = ampool.tile([128, NG, NP], bf16, name=f"adj{kt}", tag=f"adj{kt}")
            nc.sync.dma_start(out=a[:], in_=adj_d[kt])
            adj_sb.append(a)
        for kt in range(KT):
            w = ampool.tile([128, NG, NP], bf16, name=f"wg{kt}", tag=f"wg{kt}")
            nc.sync.dma_start(out=w[:], in_=wg_d[kt])
            wg_sb.append(w)

        # P^T staging: [128=(j,d), tg, t, m] bf16
        ptg = ptgpool.tile([128, NG, T, NP], bf16, name="ptg", tag="ptg")
        hb = hpool.tile([128, T, NP], bf16, name="hb", tag="hb")
        osb = osbpool.tile([128, NG, NP, T], f32, name="osb", tag="osb")

        def tmp_tile():
            return tmppool.tile([128, 2, NP], f32, name="tmp", tag="tmp")

        # alternating engine pickers for psum-reading ops (gpsimd can't)
        ps_rr = [0]

        def ps_copy_rr(out, in_):
            k = ps_rr[0] % 2
            ps_rr[0] += 1
            if k == 0:
                nc.vector.tensor_copy(out, in_)
            else:
                nc.scalar.copy(out, in_)

        # ---------------- gather machinery ----------------
        # units are generators yielding after each kt pair of MMs so the
        # gather stream can micro-interleave with weight matmuls on the PE
        # queue; psum drains are deferred and flushed at slot starts so they
        # sit at the front of the DVE/Scalar queues.
        fd = {"gen": None, "queue": [], "drains": []}

        def unit_gen(bp, t):
            pG = psA.tile([128, NG, NP], f32, name="psg", tag="ps")
            for kt in range(KT):
                for tg in range(NG):
                    nc.tensor.matmul(
                        pG[:, tg, :], g[kt][:, bp, t, :, :], wg_sb[kt][:, tg, :],
                        start=(kt == 0), stop=(kt == KT - 1),
                    )
                if kt < KT - 1:
                    yield
            fd["drains"].append((bp, t, pG))

        def take(n):
            for _ in range(n):
                if fd["gen"] is None:
                    if not fd["queue"]:
                        return
                    fd["gen"] = unit_gen(*fd["queue"].pop(0))
                try:
                    next(fd["gen"])
                except StopIteration:
                    fd["gen"] = None

        def flush_drains():
            for bp, t, pG in fd["drains"]:
                ps_copy_rr(osb[:, :, :, t], pG[:])
                if t == T - 1:
                    for tg in range(NG):
                        for i in range(N_MIX):
                            c0 = tg * C + bp * DIM
                            nc.sync.dma_start(
                                out=out_d[i][c0:c0 + DIM, :, :],
                                in_=osb[i * DIM:(i + 1) * DIM, tg, 0:N, :],
                            )
            fd["drains"].clear()

        # ---------------- layer emission ----------------
        def emit_wpair(l, p):
            t2 = slice(2 * p, 2 * p + 2)
            # weight matmuls: preacts WITHOUT bias (bias folded into the
            # elementwise chain below via relu(x+b) = max(x,-b)+b)
            pWs = {}
            for tg in range(NG):
                for blk in range(3):
                    pW = psW.tile([128, 2, NP], f32, name="psw", tag="pw")
                    lhsT = c_wblk(l, tg, blk) if blk < 2 else d_wblk(l, tg)
                    for ti in range(2):
                        nc.tensor.matmul(
                            pW[:, ti, :], lhsT, ptg[:, tg, 2 * p + ti, :],
                            start=True, stop=True,
                        )
                    pWs[(tg, blk)] = pW
                    if blk != 1:
                        take(1)
            # chain:
            #  s1 = relu(pW_c00 + bc00)                      [scalar ACT]
            #  s2 = max(pW_c01, -bc01) + s1                  [DVE STT]
            #  s3 = max(pW_c10, -bc10) + s2                  [DVE STT]
            #  s4 = max(pW_c11, -bc11) + s3                  [DVE STT]
            #  t0 = tanh(pW_d0 + bd0), t1 = tanh(pW_d1 + bd1) [scalar ACT]
            #  s5 = s4 + t0                                  [gpsimd TT]
            #  hb = (s5 + Bstar) + t1   (Bstar = bc01+bc10+bc11) [DVE STT]
            s1 = tmp_tile()
            nc.scalar.activation(s1[:], pWs[(0, 0)][:], AF.Relu, bias=cb_act(l))
            s2 = tmp_tile()
            nc.vector.scalar_tensor_tensor(
                s2[:], pWs[(0, 1)][:], cb_neg(l, 0), s1[:],
                op0=ALU.max, op1=ALU.add)
            s3 = tmp_tile()
            nc.vector.scalar_tensor_tensor(
                s3[:], pWs[(1, 0)][:], cb_neg(l, 1), s2[:],
                op0=ALU.max, op1=ALU.add)
            s4 = tmp_tile()
            nc.vector.scalar_tensor_tensor(
                s4[:], pWs[(1, 1)][:], cb_neg(l, 2), s3[:],
                op0=ALU.max, op1=ALU.add)
            t0 = tmp_tile()
            nc.scalar.activation(t0[:], pWs[(0, 2)][:], AF.Tanh, bias=d_bias(l, 0))
            t1 = tmp_tile()
            nc.scalar.activation(t1[:], pWs[(1, 2)][:], AF.Tanh, bias=d_bias(l, 1))
            # tsum runs parallel to the relu chain; hb comes one STT after s4
            tsum = tmp_tile()
            nc.gpsimd.tensor_add(tsum[:], t0[:], t1[:])
            nc.vector.scalar_tensor_tensor(
                hb[:, t2, :], s4[:], cb_star(l), tsum[:],
                op0=ALU.add, op1=ALU.add)

        def emit_layer(l):
            def transpose_half(th):
                for mt in range(KT):
                    pT = psW.tile([128, 6, 128], bf16, name="pstr", tag="pw")
                    for tt in range(6):
                        t = th * 6 + tt
                        nc.tensor.transpose(
                            pT[:, tt, :],
                            hb[:, t, mt * 128:(mt + 1) * 128],
                            ident[:],
                        )
                    dst = g[mt][:, l + 1, th * 6:(th + 1) * 6, :, :]
                    srcT = pT[:].rearrange("p t (i d) -> p t i d", i=N_MIX)
                    if mt == KT - 1:
                        # keep the virtual-node + pad rows (>=116) intact
                        nc.vector.tensor_copy(dst[0:96], srcT[0:96])
                        nc.vector.tensor_copy(dst[96:116], srcT[96:116])
                    else:
                        nc.vector.tensor_copy(dst, srcT)

            for p in range(6):
                flush_drains()
                pP = []
                for t in (2 * p, 2 * p + 1):
                    ps = psA.tile([128, NG, NP], f32, name="psadj", tag="ps")
                    for kt in range(KT):
                        for tg in range(NG):
                            nc.tensor.matmul(
                                ps[:, tg, :], g[kt][:, l, t, :, :],
                                adj_sb[kt][:, tg, :],
                                start=(kt == 0), stop=(kt == KT - 1),
                            )
                    pP.append(ps)
                for ti, t in enumerate((2 * p, 2 * p + 1)):
                    ps_copy_rr(ptg[:, :, t, :], pP[ti][:])
                if p == 5:
                    # first t-half of hb (pairs 0-2) is final: transpose early
                    transpose_half(0)
                    if l + 1 <= 2:
                        fd["queue"].extend((l + 1, t) for t in range(6))
                take(2)
                if p > 0:
                    emit_wpair(l, p - 1)
            emit_wpair(l, 5)
            flush_drains()
            take(8)
            transpose_half(1)
            if l + 1 <= 2:
                fd["queue"].extend((l + 1, t) for t in range(6, T))

        # ---------------- schedule ----------------
        fd["queue"] = [(0, t) for t in range(T)]
        emit_layer(0)
        emit_layer(1)
        while fd["queue"] or fd["gen"] is not None:
            take(4)
            flush_drains()
        flush_drains()

    _dedupe_ldweights(nc)
    nc.compile()
    _BUILT["nc"] = nc
    return nc


def _host_prep(x0, x1, graphs, neighbors, neighbors_weight, a_weight, B_weight,
               a_bias, B_bias):
    """Fuse weights, densify gather, build per-core input maps."""
    import concourse.mybir as mybir
    bf = mybir.dt.np(mybir.dt.bfloat16)
    f = np.float32
    x0 = np.asarray(x0, f)
    x1 = np.asarray(x1, f)
    graphs = np.asarray(graphs, f)
    neighbors = np.asarray(neighbors).astype(np.int64)
    neighbors_weight = np.asarray(neighbors_weight, f)
    a_weight = np.asarray(a_weight, f)
    B_weight = np.asarray(B_weight, f)
    a_bias = np.asarray(a_bias, f)
    B_bias = np.asarray(B_bias, f)

    # fused relation weights: wc/wd [R, L, D, D], bc/bd [R, L, D]
    wc = np.sum(a_weight[0] * B_weight, axis=1)
    wd = np.sum(a_weight[1] * B_weight, axis=1)
    bc = np.sum(a_bias[0] * B_bias, axis=1)
    bd = np.sum(a_bias[1] * B_bias, axis=1)
    # wmm blob: 12 blocks of [128=(j,d), 128=(i,d')].
    # c block (l,tg,j): rows j*64.. hold [wc(tg,0,j) | wc(tg,1,j)], rest zero.
    # d block (l,tg): rows 0:64 = [-wd01 | +wd10], rows 64:128 = [+wd01 | -wd10]
    wmm = np.zeros((128, 12, 128), f)
    for l in range(N_LAYERS):
        for tg in range(NG):
            for j in range(N_MIX):
                blk = (l * NG + tg) * 2 + j
                r0 = j * 64
                wmm[r0:r0 + 64, blk, 0:64] = wc[_rel(tg, 0, j), l]
                wmm[r0:r0 + 64, blk, 64:128] = wc[_rel(tg, 1, j), l]
            blk = 8 + l * NG + tg
            wd01, wd10 = wd[_rel(tg, 0, 1), l], wd[_rel(tg, 1, 0), l]
            wmm[0:64, blk, 0:64] = -wd01
            wmm[0:64, blk, 64:128] = wd10
            wmm[64:128, blk, 0:64] = wd01
            wmm[64:128, blk, 64:128] = -wd10

    bias = np.zeros((128, 16), f)
    hc = np.zeros((12, N_LAYERS, T, N_MIX, DIM), f)

    def bc_col(l, tg, j):
        v = np.zeros(128, f)
        v[0:64] = bc[_rel(tg, 0, j), l]
        v[64:128] = bc[_rel(tg, 1, j), l]
        return v

    for l in range(N_LAYERS):
        # chain term order: ACT=(tg0,j0); STT: (tg0,j1), (tg1,j0), (tg1,j1)
        bias[:, 7 * l] = bc_col(l, 0, 0)
        stt = [bc_col(l, 0, 1), bc_col(l, 1, 0), bc_col(l, 1, 1)]
        for k in range(3):
            bias[:, 7 * l + 1 + k] = -stt[k]
        bias[:, 7 * l + 4] = stt[0] + stt[1] + stt[2]
        for tg in range(NG):
            col = 7 * l + 5 + tg
            bias[0:64, col] = bd[_rel(tg, 0, 1), l]
            bias[64:128, col] = bd[_rel(tg, 1, 0), l]
        for i in range(N_MIX):
            acc = np.zeros(DIM, f)
            for tg in range(NG):
                acc += np.tanh(bd[_rel(tg, i, i), l])
            hc[0, l, :, i, :] = acc

    # adjacency / gather weights with the virtual hconst node at row N:
    # row N carries per-column sums so the constant propagates additively.
    adjp = np.zeros((NG, NP, NP), f)
    adjp[:, :N, :N] = graphs
    adjp[:, N, :N] = graphs.sum(axis=1)
    wgp = np.zeros((NG, NP, NP), f)
    for tg in range(NG):
        np.add.at(
            wgp[tg],
            (neighbors[tg].reshape(-1),
             np.repeat(np.arange(N), K)),
            neighbors_weight[tg].reshape(-1),
        )
    wgp[:, N, :N] = wgp[:, :N, :N].sum(axis=1)
    adjb = np.ascontiguousarray(
        adjp.reshape(NG, KT, 128, NP).transpose(1, 2, 0, 3)).astype(bf)
    wgb = np.ascontiguousarray(
        wgp.reshape(NG, KT, 128, NP).transpose(1, 2, 0, 3)).astype(bf)
    wmmb = wmm.astype(bf)
    hcb = hc.astype(bf)

    in_maps = []
    for b in range(NCORES):
        xn = np.zeros((NP, T, N_MIX, DIM), f)
        xn[:N, :, 0, :] = np.transpose(x0[b], (1, 2, 0))  # [D,N,T] -> [N,T,D]
        xn[:N, :, 1, :] = np.transpose(x1[b], (1, 2, 0))
        in_maps.append({
            "xn": xn.astype(bf), "adjb": adjb, "wgb": wgb, "wmm": wmmb,
            "bias": bias, "hc": hcb,
        })
    return in_maps


def kernel(x0, x1, graphs, neighbors, neighbors_weight, a_weight, B_weight,
           a_bias, B_bias):
    from concourse.bass_utils import run_bass_kernel_spmd

    nc = _build()
    in_maps = _host_prep(x0, x1, graphs, neighbors, neighbors_weight,
                         a_weight, B_weight, a_bias, B_bias)
    trace = bool(int(os.environ.get("KERNEL_TRACE", "0")))
    res = run_bass_kernel_spmd(nc, in_maps, list(range(NCORES)), trace=trace)
    kernel.last_result = res

    out0 = np.stack([res.results[b]["out0"] for b in range(NCORES)])  # [B, 384, 500, 12]
    out1 = np.stack([res.results[b]["out1"] for b in range(NCORES)])
    return out0, out1


kernel.last_result = None


# revision 13
# speedup vs baseline: 1.0421x; 1.0421x over previous
"""CMRGCN Trainium2 kernel (v3).

Sharding: data-parallel over batch B=8 across the 8 NeuronCores (core b gets
batch b). Adjacency / neighbor weights / fused relation weights are replicated.

Key structure (N=500 padded to NP=512):
  g[mt]  [128n, cb(3), T, MIX, DIM] bf16  — node-major snapshots (x, h1, h2)
  adjacency MM: lhsT = g-slice [n, (i,d)] (stationary, shared across the two
      relation graphs -> deduped weight load), moving = adj [n, m] per graph,
      psum P^T [(i,d), (tg, m)] accumulated over 4 n-chunks.
  weight MM: lhsT = fused W block [128, 128], moving = ptg [(j,d), (t, m)].
  nonlinear terms (4 relu + 2 tanh per layer) are written as bf16 tmp tiles
      (DVE tensor_scalar / Scalar ACT) and tree-summed in bf16 straight into
      hb (no f32 h tensor, no sequential accumulation chain).
  hconst (the j==i tanh(bias) constant) rides a virtual node: row N of the
      padded adjacency / gather matrices holds per-column sums, and g row N
      (partition 116 of the last node tile) is DMA-loaded with the constant
      after each layer's transposes, so the constant propagates through the
      matmuls for free.
  gather MM: lhsT = g-slice [n, (i,d)], moving = wg [n, m] (host-densified
      neighbor weights), interleaved into layer pipeline slots to keep the PE
      busy; psum drained in one [128,1024] op per (block, t) into a single
      osb [128, NG, NP, T] staging tile, DMA'd out as contiguous
      [64, 500, 12] blocks.
  A custom pass drops back-to-back redundant LDWEIGHTS (same stationary AP)
      before compile — bass emits one per matmul otherwise.
"""

import os
import numpy as np

B, T, N, DIM = 8, 12, 500, 64
N_MIX, N_LAYERS, N_HEADS, N_REL, NG, K = 2, 2, 4, 8, 2, 20
NP = 512          # padded node count
KT = NP // 128    # node tiles
C = DIM * (N_LAYERS + 1)   # 192 channels per mix in g
NCORES = 8

_BUILT = {}


def _rel(tg, i, j):
    return (tg * N_MIX + i) * N_MIX + j


def _dedupe_ldweights(nc):
    """Drop Ldweights that reload the stationary operand already in the PE
    array (identical AP, no sync side effects)."""
    ndrop = 0
    for fn in nc.m.functions:
        for blk in fn.blocks:
            insts = blk.instructions
            cur_key = None
            keep = []
            for inst in insts:
                op = inst.opcode
                if op == "Ldweights":
                    key = (str(inst.ins[0]), str(inst.is_transpose),
                           str(inst.perf_mode), str(inst.tile_position))
                    si = inst.sync_info
                    ups = list(si.on_update) if si is not None else []
                    waits = list(si.on_wait) if si is not None else []
                    if key == cur_key and not ups and not waits:
                        ndrop += 1
                        continue
                    cur_key = key
                keep.append(inst)
            if len(keep) != len(insts):
                while len(blk.instructions):
                    blk.instructions.pop()
                for inst in keep:
                    blk.instructions.append(inst)
    return ndrop


def _build():
    """Build + trace the single-core SPMD Bass program once."""
    if "nc" in _BUILT:
        return _BUILT["nc"]

    from contextlib import ExitStack
    import concourse.bass as bass
    import concourse.tile as tile
    import concourse.mybir as mybir
    from concourse import bacc
    from concourse.masks import make_identity

    f32 = mybir.dt.float32
    bf16 = mybir.dt.bfloat16
    AF = mybir.ActivationFunctionType
    ALU = mybir.AluOpType

    nc = bacc.Bacc("TRN2", target_bir_lowering=False, debug=False)

    xn_d = nc.dram_tensor("xn", [NP, T, N_MIX, DIM], bf16, kind="ExternalInput").ap()
    adj_d = nc.dram_tensor("adjb", [KT, 128, NG, NP], bf16, kind="ExternalInput").ap()
    wg_d = nc.dram_tensor("wgb", [KT, 128, NG, NP], bf16, kind="ExternalInput").ap()
    wmm_d = nc.dram_tensor("wmm", [128, 12, 128], bf16, kind="ExternalInput").ap()
    bias_d = nc.dram_tensor("bias", [128, 16], f32, kind="ExternalInput").ap()
    # virtual-node rows: partitions 116..127 of the last node tile for
    # c-blocks 1..2 (row 0 = hconst, rest zero), loaded once at startup
    hc_d = nc.dram_tensor("hc", [12, N_LAYERS, T, N_MIX, DIM], bf16,
                          kind="ExternalInput").ap()
    out_d = [
        nc.dram_tensor(f"out{i}", [NG * C, N, T], f32, kind="ExternalOutput").ap()
        for i in range(N_MIX)
    ]

    with tile.TileContext(nc) as tc, ExitStack() as ctx:
        wpool = ctx.enter_context(tc.tile_pool(name="wpool", bufs=1))
        gpool = ctx.enter_context(tc.tile_pool(name="gpool", bufs=1))
        ampool = ctx.enter_context(tc.tile_pool(name="ampool", bufs=1))
        ptgpool = ctx.enter_context(tc.tile_pool(name="ptgpool", bufs=1))
        hpool = ctx.enter_context(tc.tile_pool(name="hpool", bufs=1))
        tmppool = ctx.enter_context(tc.tile_pool(name="tmppool", bufs=12))
        osbpool = ctx.enter_context(tc.tile_pool(name="osbpool", bufs=1))
        psA = ctx.enter_context(tc.tile_pool(name="psA", bufs=2, space="PSUM"))
        psW = ctx.enter_context(tc.tile_pool(name="psW", bufs=2, space="PSUM"))

        # --- constants / weights to SBUF ---
        wmm_sb = wpool.tile([128, 12, 128], bf16, name="wmm_sb")
        nc.sync.dma_start(out=wmm_sb[:], in_=wmm_d[:])
        bias_sb = wpool.tile([128, 16], f32, name="bias_sb")
        nc.sync.dma_start(out=bias_sb[:], in_=bias_d[:])
        ident = wpool.tile([128, 128], bf16, name="ident")
        make_identity(nc, ident[:])


        def c_wblk(l, tg, j):
            return wmm_sb[:, (l * NG + tg) * 2 + j, :]

        def d_wblk(l, tg):
            return wmm_sb[:, 8 + l * NG + tg, :]

        # bias columns per layer l: 0: bc(tg0,j0) for the ACT term,
        # 1..3: negated bc for the three STT relu terms, 4: their sum (Bstar),
        # 5..6: d-path biases. Layer stride 7.
        def cb_act(l):
            return bias_sb[:, 7 * l:7 * l + 1]

        def cb_neg(l, k):
            return bias_sb[:, 7 * l + 1 + k:7 * l + 2 + k]

        def cb_star(l):
            return bias_sb[:, 7 * l + 4:7 * l + 5]

        def d_bias(l, tg):
            return bias_sb[:, 7 * l + 5 + tg:7 * l + 6 + tg]

        # --- inputs: x into g c-block 0; adjacency + gather-weight tiles ---
        g = []
        for mt in range(KT):
            gt = gpool.tile([128, 3, T, N_MIX, DIM], bf16, name=f"g{mt}", tag=f"g{mt}")
            g.append(gt)
        for mt in range(KT):
            nc.sync.dma_start(
                out=g[mt][:, 0, :, :, :],
                in_=xn_d[mt * 128:(mt + 1) * 128, :, :, :],
            )

        # virtual-node rows (node N = partition 116 of the last node tile,
        # plus zeroed pad rows) for c-blocks 1..2: written once here; the
        # transpose drains below never touch partitions >= 116 of g[3]
        nc.sync.dma_start(out=g[KT - 1][116:128, 1:3, :, :, :], in_=hc_d[:])

        adj_sb, wg_sb = [], []
        for kt in range(KT):
            a = ampool.tile([128, NG, NP], bf16, name=f"adj{kt}", tag=f"adj{kt}")
            nc.sync.dma_start(out=a[:], in_=adj_d[kt])
            adj_sb.append(a)
        for kt in range(KT):
            w = ampool.tile([128, NG, NP], bf16, name=f"wg{kt}", tag=f"wg{kt}")
            nc.sync.dma_start(out=w[:], in_=wg_d[kt])
            wg_sb.append(w)

        # P^T staging: [128=(j,d), tg, t, m] bf16
        ptg = ptgpool.tile([128, NG, T, NP], bf16, name="ptg", tag="ptg")
        hb = hpool.tile([128, T, NP], bf16, name="hb", tag="hb")
        osb = osbpool.tile([128, NG, NP, T], f32, name="osb", tag="osb")

        def tmp_tile():
            return tmppool.tile([128, 2, NP], f32, name="tmp", tag="tmp")

        # alternating engine pickers for psum-reading ops (gpsimd can't)
        ps_rr = [0]

        def ps_copy_rr(out, in_):
            k = ps_rr[0] % 2
            ps_rr[0] += 1
            if k == 0:
                nc.vector.tensor_copy(out, in_)
            else:
                nc.scalar.copy(out, in_)

        # ---------------- gather machinery ----------------
        # units are generators yielding after each kt pair of MMs so the
        # gather stream can micro-interleave with weight matmuls on the PE
        # queue; psum drains are deferred and flushed at slot starts so they
        # sit at the front of the DVE/Scalar queues.
        fd = {"gen": None, "queue": [], "drains": []}

        def unit_gen(bp, t):
            pG = psA.tile([128, NG, NP], f32, name="psg", tag="ps")
            for kt in range(KT):
                for tg in range(NG):
                    nc.tensor.matmul(
                        pG[:, tg, :], g[kt][:, bp, t, :, :], wg_sb[kt][:, tg, :],
                        start=(kt == 0), stop=(kt == KT - 1),
                    )
                if kt < KT - 1:
                    yield
            fd["drains"].append((bp, t, pG))

        def take(n):
            for _ in range(n):
                if fd["gen"] is None:
                    if not fd["queue"]:
                        return
                    fd["gen"] = unit_gen(*fd["queue"].pop(0))
                try:
                    next(fd["gen"])
                except StopIteration:
                    fd["gen"] = None

        def flush_drains():
            for bp, t, pG in fd["drains"]:
                ps_copy_rr(osb[:, :, :, t], pG[:])
                if t == T - 1:
                    for tg in range(NG):
                        for i in range(N_MIX):
                            c0 = tg * C + bp * DIM
                            nc.sync.dma_start(
                                out=out_d[i][c0:c0 + DIM, :, :],
                                in_=osb[i * DIM:(i + 1) * DIM, tg, 0:N, :],
                            )
            fd["drains"].clear()

        # ---------------- layer emission ----------------
        def emit_wpair(l, p):
            t2 = slice(2 * p, 2 * p + 2)
            # weight matmuls: preacts WITHOUT bias (bias folded into the
            # elementwise chain below via relu(x+b) = max(x,-b)+b)
            pWs = {}
            for tg in range(NG):
                for blk in range(3):
                    pW = psW.tile([128, 2, NP], f32, name="psw", tag="pw")
                    lhsT = c_wblk(l, tg, blk) if blk < 2 else d_wblk(l, tg)
                    for ti in range(2):
                        nc.tensor.matmul(
                            pW[:, ti, :], lhsT, ptg[:, tg, 2 * p + ti, :],
                            start=True, stop=True,
                        )
                    pWs[(tg, blk)] = pW
                    take(1)
            # chain:
            #  s1 = relu(pW_c00 + bc00)                      [scalar ACT]
            #  s2 = max(pW_c01, -bc01) + s1                  [DVE STT]
            #  s3 = max(pW_c10, -bc10) + s2                  [DVE STT]
            #  s4 = max(pW_c11, -bc11) + s3                  [DVE STT]
            #  t0 = tanh(pW_d0 + bd0), t1 = tanh(pW_d1 + bd1) [scalar ACT]
            #  s5 = s4 + t0                                  [gpsimd TT]
            #  hb = (s5 + Bstar) + t1   (Bstar = bc01+bc10+bc11) [DVE STT]
            s1 = tmp_tile()
            nc.scalar.activation(s1[:], pWs[(0, 0)][:], AF.Relu, bias=cb_act(l))
            s2 = tmp_tile()
            nc.vector.scalar_tensor_tensor(
                s2[:], pWs[(0, 1)][:], cb_neg(l, 0), s1[:],
                op0=ALU.max, op1=ALU.add)
            s3 = tmp_tile()
            nc.vector.scalar_tensor_tensor(
                s3[:], pWs[(1, 0)][:], cb_neg(l, 1), s2[:],
                op0=ALU.max, op1=ALU.add)
            s4 = tmp_tile()
            nc.vector.scalar_tensor_tensor(
                s4[:], pWs[(1, 1)][:], cb_neg(l, 2), s3[:],
                op0=ALU.max, op1=ALU.add)
            t0 = tmp_tile()
            nc.scalar.activation(t0[:], pWs[(0, 2)][:], AF.Tanh, bias=d_bias(l, 0))
            t1 = tmp_tile()
            nc.scalar.activation(t1[:], pWs[(1, 2)][:], AF.Tanh, bias=d_bias(l, 1))
            # tsum runs parallel to the relu chain; hb comes one STT after s4
            tsum = tmp_tile()
            nc.gpsimd.tensor_add(tsum[:], t0[:], t1[:])
            nc.vector.scalar_tensor_tensor(
                hb[:, t2, :], s4[:], cb_star(l), tsum[:],
                op0=ALU.add, op1=ALU.add)

        def emit_layer(l):
            def transpose_half(th):
                for mt in range(KT):
                    pT = psW.tile([128, 6, 128], bf16, name="pstr", tag="pw")
                    for tt in range(6):
                        t = th * 6 + tt
                        nc.tensor.transpose(
                            pT[:, tt, :],
                            hb[:, t, mt * 128:(mt + 1) * 128],
                            ident[:],
                        )
                    dst = g[mt][:, l + 1, th * 6:(th + 1) * 6, :, :]
                    srcT = pT[:].rearrange("p t (i d) -> p t i d", i=N_MIX)
                    if mt == KT - 1:
                        # keep the virtual-node + pad rows (>=116) intact
                        nc.vector.tensor_copy(dst[0:96], srcT[0:96])
                        nc.vector.tensor_copy(dst[96:116], srcT[96:116])
                    else:
                        nc.vector.tensor_copy(dst, srcT)

            for p in range(6):
                flush_drains()
                pP = []
                for t in (2 * p, 2 * p + 1):
                    ps = psA.tile([128, NG, NP], f32, name="psadj", tag="ps")
                    for kt in range(KT):
                        for tg in range(NG):
                            nc.tensor.matmul(
                                ps[:, tg, :], g[kt][:, l, t, :, :],
                                adj_sb[kt][:, tg, :],
                                start=(kt == 0), stop=(kt == KT - 1),
                            )
                    pP.append(ps)
                for ti, t in enumerate((2 * p, 2 * p + 1)):
                    ps_copy_rr(ptg[:, :, t, :], pP[ti][:])
                if p == 5:
                    # first t-half of hb (pairs 0-2) is final: transpose early
                    transpose_half(0)
                    if l + 1 <= 2:
                        fd["queue"].extend((l + 1, t) for t in range(6))
                take(4)
                if p > 0:
                    emit_wpair(l, p - 1)
            emit_wpair(l, 5)
            flush_drains()
            take(8)
            transpose_half(1)
            if l + 1 <= 2:
                fd["queue"].extend((l + 1, t) for t in range(6, T))

        # ---------------- schedule ----------------
        fd["queue"] = [(0, t) for t in range(T)]
        emit_layer(0)
        emit_layer(1)
        while fd["queue"] or fd["gen"] is not None:
            take(4)
            flush_drains()
        flush_drains()

    _dedupe_ldweights(nc)
    nc.compile()
    _BUILT["nc"] = nc
    return nc


def _host_prep(x0, x1, graphs, neighbors, neighbors_weight, a_weight, B_weight,
               a_bias, B_bias):
    """Fuse weights, densify gather, build per-core input maps."""
    import concourse.mybir as mybir
    bf = mybir.dt.np(mybir.dt.bfloat16)
    f = np.float32
    x0 = np.asarray(x0, f)
    x1 = np.asarray(x1, f)
    graphs = np.asarray(graphs, f)
    neighbors = np.asarray(neighbors).astype(np.int64)
    neighbors_weight = np.asarray(neighbors_weight, f)
    a_weight = np.asarray(a_weight, f)
    B_weight = np.asarray(B_weight, f)
    a_bias = np.asarray(a_bias, f)
    B_bias = np.asarray(B_bias, f)

    # fused relation weights: wc/wd [R, L, D, D], bc/bd [R, L, D]
    wc = np.sum(a_weight[0] * B_weight, axis=1)
    wd = np.sum(a_weight[1] * B_weight, axis=1)
    bc = np.sum(a_bias[0] * B_bias, axis=1)
    bd = np.sum(a_bias[1] * B_bias, axis=1)
    # wmm blob: 12 blocks of [128=(j,d), 128=(i,d')].
    # c block (l,tg,j): rows j*64.. hold [wc(tg,0,j) | wc(tg,1,j)], rest zero.
    # d block (l,tg): rows 0:64 = [-wd01 | +wd10], rows 64:128 = [+wd01 | -wd10]
    wmm = np.zeros((128, 12, 128), f)
    for l in range(N_LAYERS):
        for tg in range(NG):
            for j in range(N_MIX):
                blk = (l * NG + tg) * 2 + j
                r0 = j * 64
                wmm[r0:r0 + 64, blk, 0:64] = wc[_rel(tg, 0, j), l]
                wmm[r0:r0 + 64, blk, 64:128] = wc[_rel(tg, 1, j), l]
            blk = 8 + l * NG + tg
            wd01, wd10 = wd[_rel(tg, 0, 1), l], wd[_rel(tg, 1, 0), l]
            wmm[0:64, blk, 0:64] = -wd01
            wmm[0:64, blk, 64:128] = wd10
            wmm[64:128, blk, 0:64] = wd01
            wmm[64:128, blk, 64:128] = -wd10

    bias = np.zeros((128, 16), f)
    hc = np.zeros((12, N_LAYERS, T, N_MIX, DIM), f)

    def bc_col(l, tg, j):
        v = np.zeros(128, f)
        v[0:64] = bc[_rel(tg, 0, j), l]
        v[64:128] = bc[_rel(tg, 1, j), l]
        return v

    for l in range(N_LAYERS):
        # chain term order: ACT=(tg0,j0); STT: (tg0,j1), (tg1,j0), (tg1,j1)
        bias[:, 7 * l] = bc_col(l, 0, 0)
        stt = [bc_col(l, 0, 1), bc_col(l, 1, 0), bc_col(l, 1, 1)]
        for k in range(3):
            bias[:, 7 * l + 1 + k] = -stt[k]
        bias[:, 7 * l + 4] = stt[0] + stt[1] + stt[2]
        for tg in range(NG):
            col = 7 * l + 5 + tg
            bias[0:64, col] = bd[_rel(tg, 0, 1), l]
            bias[64:128, col] = bd[_rel(tg, 1, 0), l]
        for i in range(N_MIX):
            acc = np.zeros(DIM, f)
            for tg in range(NG):
                acc += np.tanh(bd[_rel(tg, i, i), l])
            hc[0, l, :, i, :] = acc

    # adjacency / gather weights with the virtual hconst node at row N:
    # row N carries per-column sums so the constant propagates additively.
    adjp = np.zeros((NG, NP, NP), f)
    adjp[:, :N, :N] = graphs
    adjp[:, N, :N] = graphs.sum(axis=1)
    wgp = np.zeros((NG, NP, NP), f)
    for tg in range(NG):
        np.add.at(
            wgp[tg],
            (neighbors[tg].reshape(-1),
             np.repeat(np.arange(N), K)),
            neighbors_weight[tg].reshape(-1),
        )
    wgp[:, N, :N] = wgp[:, :N, :N].sum(axis=1)
    adjb = np.ascontiguousarray(
        adjp.reshape(NG, KT, 128, NP).transpose(1, 2, 0, 3)).astype(bf)
    wgb = np.ascontiguousarray(
        wgp.reshape(NG, KT, 128, NP).transpose(1, 2, 0, 3)).astype(bf)
    wmmb = wmm.astype(bf)
    hcb = hc.astype(bf)

    in_maps = []
    for b in range(NCORES):
        xn = np.zeros((NP, T, N_MIX, DIM), f)
        xn[:N, :, 0, :] = np.transpose(x0[b], (1, 2, 0))  # [D,N,T] -> [N,T,D]
        xn[:N, :, 1, :] = np.transpose(x1[b], (1, 2, 0))
        in_maps.append({
            "xn": xn.astype(bf), "adjb": adjb, "wgb": wgb, "wmm": wmmb,
            "bias": bias, "hc": hcb,
        })
    return in_maps


def kernel(x0, x1, graphs, neighbors, neighbors_weight, a_weight, B_weight,
           a_bias, B_bias):
    from concourse.bass_utils import run_bass_kernel_spmd

    nc = _build()
    in_maps = _host_prep(x0, x1, graphs, neighbors, neighbors_weight,
                         a_weight, B_weight, a_bias, B_bias)
    trace = bool(int(os.environ.get("KERNEL_TRACE", "0")))
    res = run_bass_kernel_spmd(nc, in_maps, list(range(NCORES)), trace=trace)
    kernel.last_result = res

    out0 = np.stack([res.results[b]["out0"] for b in range(NCORES)])  # [B, 384, 500, 12]
    out1 = np.stack([res.results[b]["out1"] for b in range(NCORES)])
    return out0, out1


kernel.last_result = None


# revision 15
# speedup vs baseline: 1.1541x; 1.1074x over previous
"""CMRGCN Trainium2 kernel (v3).

Sharding: data-parallel over batch B=8 across the 8 NeuronCores (core b gets
batch b). Adjacency / neighbor weights / fused relation weights are replicated.

Key structure (N=500 padded to NP=512):
  g[mt]  [128n, cb(3), T, MIX, DIM] bf16  — node-major snapshots (x, h1, h2)
  adjacency MM: lhsT = g-slice [n, (i,d)] (stationary, shared across the two
      relation graphs -> deduped weight load), moving = adj [n, m] per graph,
      psum P^T [(i,d), (tg, m)] accumulated over 4 n-chunks.
  weight MM: lhsT = fused W block [128, 128], moving = ptg [(j,d), (t, m)].
  nonlinear terms (4 relu + 2 tanh per layer) are written as bf16 tmp tiles
      (DVE tensor_scalar / Scalar ACT) and tree-summed in bf16 straight into
      hb (no f32 h tensor, no sequential accumulation chain).
  hconst (the j==i tanh(bias) constant) rides a virtual node: row N of the
      padded adjacency / gather matrices holds per-column sums, and g row N
      (partition 116 of the last node tile) is DMA-loaded with the constant
      after each layer's transposes, so the constant propagates through the
      matmuls for free.
  gather MM: lhsT = g-slice [n, (i,d)], moving = wg [n, m] (host-densified
      neighbor weights), interleaved into layer pipeline slots to keep the PE
      busy; psum drained in one [128,1024] op per (block, t) into a single
      osb [128, NG, NP, T] staging tile, DMA'd out as contiguous
      [64, 500, 12] blocks.
  A custom pass drops back-to-back redundant LDWEIGHTS (same stationary AP)
      before compile — bass emits one per matmul otherwise.
"""

import os
import numpy as np

B, T, N, DIM = 8, 12, 500, 64
N_MIX, N_LAYERS, N_HEADS, N_REL, NG, K = 2, 2, 4, 8, 2, 20
NP = 512          # padded node count
KT = NP // 128    # node tiles
C = DIM * (N_LAYERS + 1)   # 192 channels per mix in g
NCORES = 8

_BUILT = {}


def _rel(tg, i, j):
    return (tg * N_MIX + i) * N_MIX + j


def _dedupe_ldweights(nc):
    """Drop Ldweights that reload the stationary operand already in the PE
    array (identical AP, no sync side effects)."""
    ndrop = 0
    for fn in nc.m.functions:
        for blk in fn.blocks:
            insts = blk.instructions
            cur_key = None
            keep = []
            for inst in insts:
                op = inst.opcode
                if op == "Ldweights":
                    key = (str(inst.ins[0]), str(inst.is_transpose),
                           str(inst.perf_mode), str(inst.tile_position))
                    si = inst.sync_info
                    ups = list(si.on_update) if si is not None else []
                    waits = list(si.on_wait) if si is not None else []
                    if key == cur_key and not ups and not waits:
                        ndrop += 1
                        continue
                    cur_key = key
                keep.append(inst)
            if len(keep) != len(insts):
                while len(blk.instructions):
                    blk.instructions.pop()
                for inst in keep:
                    blk.instructions.append(inst)
    return ndrop


def _build():
    """Build + trace the single-core SPMD Bass program once."""
    if "nc" in _BUILT:
        return _BUILT["nc"]

    from contextlib import ExitStack
    import concourse.bass as bass
    import concourse.tile as tile
    import concourse.mybir as mybir
    from concourse import bacc
    from concourse.masks import make_identity

    f32 = mybir.dt.float32
    bf16 = mybir.dt.bfloat16
    AF = mybir.ActivationFunctionType
    ALU = mybir.AluOpType

    nc = bacc.Bacc("TRN2", target_bir_lowering=False, debug=False)

    xn_d = nc.dram_tensor("xn", [NP, T, N_MIX, DIM], bf16, kind="ExternalInput").ap()
    adj_d = nc.dram_tensor("adjb", [KT, 128, NG, NP], bf16, kind="ExternalInput").ap()
    wg_d = nc.dram_tensor("wgb", [KT, 128, NG, NP], bf16, kind="ExternalInput").ap()
    wmm_d = nc.dram_tensor("wmm", [128, 12, 128], bf16, kind="ExternalInput").ap()
    bias_d = nc.dram_tensor("bias", [128, 16], f32, kind="ExternalInput").ap()
    # virtual-node rows: partitions 116..127 of the last node tile for
    # c-blocks 1..2 (row 0 = hconst, rest zero), loaded once at startup
    hc_d = nc.dram_tensor("hc", [12, N_LAYERS, T, N_MIX, DIM], bf16,
                          kind="ExternalInput").ap()
    out_d = [
        nc.dram_tensor(f"out{i}", [NG * C, N, T], f32, kind="ExternalOutput").ap()
        for i in range(N_MIX)
    ]

    with tile.TileContext(nc) as tc, ExitStack() as ctx:
        wpool = ctx.enter_context(tc.tile_pool(name="wpool", bufs=1))
        gpool = ctx.enter_context(tc.tile_pool(name="gpool", bufs=1))
        ampool = ctx.enter_context(tc.tile_pool(name="ampool", bufs=1))
        ptgpool = ctx.enter_context(tc.tile_pool(name="ptgpool", bufs=1))
        hpool = ctx.enter_context(tc.tile_pool(name="hpool", bufs=1))
        tmppool = ctx.enter_context(tc.tile_pool(name="tmppool", bufs=5))
        osbpool = ctx.enter_context(tc.tile_pool(name="osbpool", bufs=2))
        psA = ctx.enter_context(tc.tile_pool(name="psA", bufs=2, space="PSUM"))
        psW = ctx.enter_context(tc.tile_pool(name="psW", bufs=2, space="PSUM"))

        # --- constants / weights to SBUF ---
        wmm_sb = wpool.tile([128, 12, 128], bf16, name="wmm_sb")
        nc.sync.dma_start(out=wmm_sb[:], in_=wmm_d[:])
        bias_sb = wpool.tile([128, 16], f32, name="bias_sb")
        nc.sync.dma_start(out=bias_sb[:], in_=bias_d[:])
        ident = wpool.tile([128, 128], bf16, name="ident")
        make_identity(nc, ident[:])


        def c_wblk(l, tg, j):
            return wmm_sb[:, (l * NG + tg) * 2 + j, :]

        def d_wblk(l, tg):
            return wmm_sb[:, 8 + l * NG + tg, :]

        # bias columns per layer l: 0: bc(tg0,j0) for the ACT term,
        # 1..3: negated bc for the three STT relu terms, 4: their sum (Bstar),
        # 5..6: d-path biases. Layer stride 7.
        def cb_act(l):
            return bias_sb[:, 7 * l:7 * l + 1]

        def cb_neg(l, k):
            return bias_sb[:, 7 * l + 1 + k:7 * l + 2 + k]

        def cb_star(l):
            return bias_sb[:, 7 * l + 4:7 * l + 5]

        def d_bias(l, tg):
            return bias_sb[:, 7 * l + 5 + tg:7 * l + 6 + tg]

        # --- inputs: x into g c-block 0; adjacency + gather-weight tiles ---
        g = []
        for mt in range(KT):
            gt = gpool.tile([128, 3, T, N_MIX, DIM], bf16, name=f"g{mt}", tag=f"g{mt}")
            g.append(gt)
        for mt in range(KT):
            nc.sync.dma_start(
                out=g[mt][:, 0, :, :, :],
                in_=xn_d[mt * 128:(mt + 1) * 128, :, :, :],
            )

        # virtual-node rows (node N = partition 116 of the last node tile,
        # plus zeroed pad rows) for c-blocks 1..2: written once here; the
        # transpose drains below never touch partitions >= 116 of g[3]
        nc.sync.dma_start(out=g[KT - 1][116:128, 1:3, :, :, :], in_=hc_d[:])

        adj_sb, wg_sb = [], []
        for kt in range(KT):
            a = ampool.tile([128, NG, NP], bf16, name=f"adj{kt}", tag=f"adj{kt}")
            nc.sync.dma_start(out=a[:], in_=adj_d[kt])
            adj_sb.append(a)
        for kt in range(KT):
            w = ampool.tile([128, NG, NP], bf16, name=f"wg{kt}", tag=f"wg{kt}")
            nc.sync.dma_start(out=w[:], in_=wg_d[kt])
            wg_sb.append(w)

        # P^T staging: [128=(j,d), tg, t, m] bf16
        ptg = ptgpool.tile([128, NG, T, NP], bf16, name="ptg", tag="ptg")
        hb = hpool.tile([128, T, NP], bf16, name="hb", tag="hb")

        def tmp_tile():
            return tmppool.tile([128, 2, NP], f32, name="tmp", tag="tmp")

        # alternating engine pickers for psum-reading ops (gpsimd can't)
        ps_rr = [0]

        def ps_copy_rr(out, in_):
            k = ps_rr[0] % 2
            ps_rr[0] += 1
            if k == 0:
                nc.vector.tensor_copy(out, in_)
            else:
                nc.scalar.copy(out, in_)

        # ---------------- gather machinery ----------------
        # units are generators yielding after each kt pair of MMs so the
        # gather stream can micro-interleave with weight matmuls on the PE
        # queue; psum drains are deferred and flushed at slot starts so they
        # sit at the front of the DVE/Scalar queues.
        fd = {"gen": None, "queue": [], "drains": [], "osb": {}}

        def unit_gen(bp, t):
            if bp not in fd["osb"]:
                fd["osb"][bp] = osbpool.tile(
                    [128, NG, NP, T], f32, name=f"osb{bp}", tag="osb")
            pG = psA.tile([128, NG, NP], f32, name="psg", tag="ps")
            for kt in range(KT):
                for tg in range(NG):
                    nc.tensor.matmul(
                        pG[:, tg, :], g[kt][:, bp, t, :, :], wg_sb[kt][:, tg, :],
                        start=(kt == 0), stop=(kt == KT - 1),
                    )
                if kt < KT - 1:
                    yield
            fd["drains"].append((bp, t, pG))

        def take(n):
            for _ in range(n):
                if fd["gen"] is None:
                    if not fd["queue"]:
                        return
                    fd["gen"] = unit_gen(*fd["queue"].pop(0))
                try:
                    next(fd["gen"])
                except StopIteration:
                    fd["gen"] = None

        def flush_drains():
            for bp, t, pG in fd["drains"]:
                osb = fd["osb"][bp]
                ps_copy_rr(osb[:, :, :, t], pG[:])
                if t == T - 1:
                    for tg in range(NG):
                        for i in range(N_MIX):
                            c0 = tg * C + bp * DIM
                            nc.sync.dma_start(
                                out=out_d[i][c0:c0 + DIM, :, :],
                                in_=osb[i * DIM:(i + 1) * DIM, tg, 0:N, :],
                            )
            fd["drains"].clear()

        # ---------------- layer emission ----------------
        def emit_wpair(l, p):
            t2 = slice(2 * p, 2 * p + 2)
            # weight matmuls: preacts WITHOUT bias (bias folded into the
            # elementwise chain below via relu(x+b) = max(x,-b)+b)
            pWs = {}
            for tg in range(NG):
                for blk in range(3):
                    pW = psW.tile([128, 2, NP], f32, name="psw", tag="pw")
                    lhsT = c_wblk(l, tg, blk) if blk < 2 else d_wblk(l, tg)
                    for ti in range(2):
                        nc.tensor.matmul(
                            pW[:, ti, :], lhsT, ptg[:, tg, 2 * p + ti, :],
                            start=True, stop=True,
                        )
                    pWs[(tg, blk)] = pW
                    take(1)
            # chain:
            #  s1 = relu(pW_c00 + bc00)                      [scalar ACT]
            #  s2 = max(pW_c01, -bc01) + s1                  [DVE STT]
            #  s3 = max(pW_c10, -bc10) + s2                  [DVE STT]
            #  s4 = max(pW_c11, -bc11) + s3                  [DVE STT]
            #  t0 = tanh(pW_d0 + bd0), t1 = tanh(pW_d1 + bd1) [scalar ACT]
            #  s5 = s4 + t0                                  [gpsimd TT]
            #  hb = (s5 + Bstar) + t1   (Bstar = bc01+bc10+bc11) [DVE STT]
            s1 = tmp_tile()
            nc.scalar.activation(s1[:], pWs[(0, 0)][:], AF.Relu, bias=cb_act(l))
            s2 = tmp_tile()
            nc.vector.scalar_tensor_tensor(
                s2[:], pWs[(0, 1)][:], cb_neg(l, 0), s1[:],
                op0=ALU.max, op1=ALU.add)
            s3 = tmp_tile()
            nc.vector.scalar_tensor_tensor(
                s3[:], pWs[(1, 0)][:], cb_neg(l, 1), s2[:],
                op0=ALU.max, op1=ALU.add)
            s4 = tmp_tile()
            nc.vector.scalar_tensor_tensor(
                s4[:], pWs[(1, 1)][:], cb_neg(l, 2), s3[:],
                op0=ALU.max, op1=ALU.add)
            t0 = tmp_tile()
            nc.scalar.activation(t0[:], pWs[(0, 2)][:], AF.Tanh, bias=d_bias(l, 0))
            t1 = tmp_tile()
            nc.scalar.activation(t1[:], pWs[(1, 2)][:], AF.Tanh, bias=d_bias(l, 1))
            # tsum runs parallel to the relu chain; hb comes one STT after s4
            tsum = tmp_tile()
            nc.gpsimd.tensor_add(tsum[:], t0[:], t1[:])
            nc.vector.scalar_tensor_tensor(
                hb[:, t2, :], s4[:], cb_star(l), tsum[:],
                op0=ALU.add, op1=ALU.add)

        def emit_layer(l):
            def transpose_half(th):
                for mt in range(KT):
                    pT = psW.tile([128, 6, 128], bf16, name="pstr", tag="pw")
                    for tt in range(6):
                        t = th * 6 + tt
                        nc.tensor.transpose(
                            pT[:, tt, :],
                            hb[:, t, mt * 128:(mt + 1) * 128],
                            ident[:],
                        )
                    dst = g[mt][:, l + 1, th * 6:(th + 1) * 6, :, :]
                    srcT = pT[:].rearrange("p t (i d) -> p t i d", i=N_MIX)
                    if mt == KT - 1:
                        # keep the virtual-node + pad rows (>=116) intact
                        nc.vector.tensor_copy(dst[0:96], srcT[0:96])
                        nc.vector.tensor_copy(dst[96:116], srcT[96:116])
                    else:
                        nc.vector.tensor_copy(dst, srcT)

            for p in range(6):
                flush_drains()
                pP = []
                for t in (2 * p, 2 * p + 1):
                    ps = psA.tile([128, NG, NP], f32, name="psadj", tag="ps")
                    for kt in range(KT):
                        for tg in range(NG):
                            nc.tensor.matmul(
                                ps[:, tg, :], g[kt][:, l, t, :, :],
                                adj_sb[kt][:, tg, :],
                                start=(kt == 0), stop=(kt == KT - 1),
                            )
                    pP.append(ps)
                for ti, t in enumerate((2 * p, 2 * p + 1)):
                    ps_copy_rr(ptg[:, :, t, :], pP[ti][:])
                if p == 5:
                    # first t-half of hb (pairs 0-2) is final: transpose early
                    transpose_half(0)
                    if l + 1 <= 2:
                        fd["queue"].extend((l + 1, t) for t in range(6))
                take(4)
                if p > 0:
                    emit_wpair(l, p - 1)
            emit_wpair(l, 5)
            flush_drains()
            take(8)
            transpose_half(1)
            if l + 1 <= 2:
                fd["queue"].extend((l + 1, t) for t in range(6, T))

        # ---------------- schedule ----------------
        fd["queue"] = [(0, t) for t in range(T)]
        emit_layer(0)
        emit_layer(1)
        while fd["queue"] or fd["gen"] is not None:
            take(4)
            flush_drains()
        flush_drains()

    _dedupe_ldweights(nc)
    nc.compile()
    _BUILT["nc"] = nc
    return nc


def _host_prep(x0, x1, graphs, neighbors, neighbors_weight, a_weight, B_weight,
               a_bias, B_bias):
    """Fuse weights, densify gather, build per-core input maps."""
    import concourse.mybir as mybir
    bf = mybir.dt.np(mybir.dt.bfloat16)
    f = np.float32
    x0 = np.asarray(x0, f)
    x1 = np.asarray(x1, f)
    graphs = np.asarray(graphs, f)
    neighbors = np.asarray(neighbors).astype(np.int64)
    neighbors_weight = np.asarray(neighbors_weight, f)
    a_weight = np.asarray(a_weight, f)
    B_weight = np.asarray(B_weight, f)
    a_bias = np.asarray(a_bias, f)
    B_bias = np.asarray(B_bias, f)

    # fused relation weights: wc/wd [R, L, D, D], bc/bd [R, L, D]
    wc = np.sum(a_weight[0] * B_weight, axis=1)
    wd = np.sum(a_weight[1] * B_weight, axis=1)
    bc = np.sum(a_bias[0] * B_bias, axis=1)
    bd = np.sum(a_bias[1] * B_bias, axis=1)
    # wmm blob: 12 blocks of [128=(j,d), 128=(i,d')].
    # c block (l,tg,j): rows j*64.. hold [wc(tg,0,j) | wc(tg,1,j)], rest zero.
    # d block (l,tg): rows 0:64 = [-wd01 | +wd10], rows 64:128 = [+wd01 | -wd10]
    wmm = np.zeros((128, 12, 128), f)
    for l in range(N_LAYERS):
        for tg in range(NG):
            for j in range(N_MIX):
                blk = (l * NG + tg) * 2 + j
                r0 = j * 64
                wmm[r0:r0 + 64, blk, 0:64] = wc[_rel(tg, 0, j), l]
                wmm[r0:r0 + 64, blk, 64:128] = wc[_rel(tg, 1, j), l]
            blk = 8 + l * NG + tg
            wd01, wd10 = wd[_rel(tg, 0, 1), l], wd[_rel(tg, 1, 0), l]
            wmm[0:64, blk, 0:64] = -wd01
            wmm[0:64, blk, 64:128] = wd10
            wmm[64:128, blk, 0:64] = wd01
            wmm[64:128, blk, 64:128] = -wd10

    bias = np.zeros((128, 16), f)
    hc = np.zeros((12, N_LAYERS, T, N_MIX, DIM), f)

    def bc_col(l, tg, j):
        v = np.zeros(128, f)
        v[0:64] = bc[_rel(tg, 0, j), l]
        v[64:128] = bc[_rel(tg, 1, j), l]
        return v

    for l in range(N_LAYERS):
        # chain term order: ACT=(tg0,j0); STT: (tg0,j1), (tg1,j0), (tg1,j1)
        bias[:, 7 * l] = bc_col(l, 0, 0)
        stt = [bc_col(l, 0, 1), bc_col(l, 1, 0), bc_col(l, 1, 1)]
        for k in range(3):
            bias[:, 7 * l + 1 + k] = -stt[k]
        bias[:, 7 * l + 4] = stt[0] + stt[1] + stt[2]
        for tg in range(NG):
            col = 7 * l + 5 + tg
            bias[0:64, col] = bd[_rel(tg, 0, 1), l]
            bias[64:128, col] = bd[_rel(tg, 1, 0), l]
        for i in range(N_MIX):
            acc = np.zeros(DIM, f)
            for tg in range(NG):
                acc += np.tanh(bd[_rel(tg, i, i), l])
            hc[0, l, :, i, :] = acc

    # adjacency / gather weights with the virtual hconst node at row N:
    # row N carries per-column sums so the constant propagates additively.
    adjp = np.zeros((NG, NP, NP), f)
    adjp[:, :N, :N] = graphs
    adjp[:, N, :N] = graphs.sum(axis=1)
    wgp = np.zeros((NG, NP, NP), f)
    for tg in range(NG):
        np.add.at(
            wgp[tg],
            (neighbors[tg].reshape(-1),
             np.repeat(np.arange(N), K)),
            neighbors_weight[tg].reshape(-1),
        )
    wgp[:, N, :N] = wgp[:, :N, :N].sum(axis=1)
    adjb = np.ascontiguousarray(
        adjp.reshape(NG, KT, 128, NP).transpose(1, 2, 0, 3)).astype(bf)
    wgb = np.ascontiguousarray(
        wgp.reshape(NG, KT, 128, NP).transpose(1, 2, 0, 3)).astype(bf)
    wmmb = wmm.astype(bf)
    hcb = hc.astype(bf)

    in_maps = []
    for b in range(NCORES):
        xn = np.zeros((NP, T, N_MIX, DIM), f)
        xn[:N, :, 0, :] = np.transpose(x0[b], (1, 2, 0))  # [D,N,T] -> [N,T,D]
        xn[:N, :, 1, :] = np.transpose(x1[b], (1, 2, 0))
        in_maps.append({
            "xn": xn.astype(bf), "adjb": adjb, "wgb": wgb, "wmm": wmmb,
            "bias": bias, "hc": hcb,
        })
    return in_maps


def kernel(x0, x1, graphs, neighbors, neighbors_weight, a_weight, B_weight,
           a_bias, B_bias):
    from concourse.bass_utils import run_bass_kernel_spmd

    nc = _build()
    in_maps = _host_prep(x0, x1, graphs, neighbors, neighbors_weight,
                         a_weight, B_weight, a_bias, B_bias)
    trace = bool(int(os.environ.get("KERNEL_TRACE", "0")))
    res = run_bass_kernel_spmd(nc, in_maps, list(range(NCORES)), trace=trace)
    kernel.last_result = res

    out0 = np.stack([res.results[b]["out0"] for b in range(NCORES)])  # [B, 384, 500, 12]
    out1 = np.stack([res.results[b]["out1"] for b in range(NCORES)])
    return out0, out1


kernel.last_result = None


# revision 16
# speedup vs baseline: 1.1834x; 1.0254x over previous
"""CMRGCN Trainium2 kernel (v3).

Sharding: data-parallel over batch B=8 across the 8 NeuronCores (core b gets
batch b). Adjacency / neighbor weights / fused relation weights are replicated.

Key structure (N=500 padded to NP=512):
  g[mt]  [128n, cb(3), T, MIX, DIM] bf16  — node-major snapshots (x, h1, h2)
  adjacency MM: lhsT = g-slice [n, (i,d)] (stationary, shared across the two
      relation graphs -> deduped weight load), moving = adj [n, m] per graph,
      psum P^T [(i,d), (tg, m)] accumulated over 4 n-chunks.
  weight MM: lhsT = fused W block [128, 128], moving = ptg [(j,d), (t, m)].
  nonlinear terms (4 relu + 2 tanh per layer) are written as bf16 tmp tiles
      (DVE tensor_scalar / Scalar ACT) and tree-summed in bf16 straight into
      hb (no f32 h tensor, no sequential accumulation chain).
  hconst (the j==i tanh(bias) constant) rides a virtual node: row N of the
      padded adjacency / gather matrices holds per-column sums, and g row N
      (partition 116 of the last node tile) is DMA-loaded with the constant
      after each layer's transposes, so the constant propagates through the
      matmuls for free.
  gather MM: lhsT = g-slice [n, (i,d)], moving = wg [n, m] (host-densified
      neighbor weights), interleaved into layer pipeline slots to keep the PE
      busy; psum drained in one [128,1024] op per (block, t) into a single
      osb [128, NG, NP, T] staging tile, DMA'd out as contiguous
      [64, 500, 12] blocks.
  A custom pass drops back-to-back redundant LDWEIGHTS (same stationary AP)
      before compile — bass emits one per matmul otherwise.
"""

import os
import numpy as np

B, T, N, DIM = 8, 12, 500, 64
N_MIX, N_LAYERS, N_HEADS, N_REL, NG, K = 2, 2, 4, 8, 2, 20
NP = 512          # padded node count
KT = NP // 128    # node tiles
C = DIM * (N_LAYERS + 1)   # 192 channels per mix in g
NCORES = 8

_BUILT = {}


def _rel(tg, i, j):
    return (tg * N_MIX + i) * N_MIX + j


def _dedupe_ldweights(nc):
    """Drop Ldweights that reload the stationary operand already in the PE
    array (identical AP, no sync side effects)."""
    ndrop = 0
    for fn in nc.m.functions:
        for blk in fn.blocks:
            insts = blk.instructions
            cur_key = None
            keep = []
            for inst in insts:
                op = inst.opcode
                if op == "Ldweights":
                    key = (str(inst.ins[0]), str(inst.is_transpose),
                           str(inst.perf_mode), str(inst.tile_position))
                    si = inst.sync_info
                    ups = list(si.on_update) if si is not None else []
                    waits = list(si.on_wait) if si is not None else []
                    if key == cur_key and not ups and not waits:
                        ndrop += 1
                        continue
                    cur_key = key
                keep.append(inst)
            if len(keep) != len(insts):
                while len(blk.instructions):
                    blk.instructions.pop()
                for inst in keep:
                    blk.instructions.append(inst)
    return ndrop


def _build():
    """Build + trace the single-core SPMD Bass program once."""
    if "nc" in _BUILT:
        return _BUILT["nc"]

    from contextlib import ExitStack
    import concourse.bass as bass
    import concourse.tile as tile
    import concourse.mybir as mybir
    from concourse import bacc
    from concourse.masks import make_identity

    f32 = mybir.dt.float32
    bf16 = mybir.dt.bfloat16
    AF = mybir.ActivationFunctionType
    ALU = mybir.AluOpType

    nc = bacc.Bacc("TRN2", target_bir_lowering=False, debug=False)

    xn_d = nc.dram_tensor("xn", [NP, T, N_MIX, DIM], bf16, kind="ExternalInput").ap()
    adj_d = nc.dram_tensor("adjb", [KT, 128, NG, NP], bf16, kind="ExternalInput").ap()
    wg_d = nc.dram_tensor("wgb", [KT, 128, NG, NP], bf16, kind="ExternalInput").ap()
    wmm_d = nc.dram_tensor("wmm", [128, 12, 128], bf16, kind="ExternalInput").ap()
    bias_d = nc.dram_tensor("bias", [128, 16], f32, kind="ExternalInput").ap()
    # virtual-node rows: partitions 116..127 of the last node tile for
    # c-blocks 1..2 (row 0 = hconst, rest zero), loaded once at startup
    hc_d = nc.dram_tensor("hc", [12, N_LAYERS, T, N_MIX, DIM], bf16,
                          kind="ExternalInput").ap()
    out_d = [
        nc.dram_tensor(f"out{i}", [NG * C, N, T], f32, kind="ExternalOutput").ap()
        for i in range(N_MIX)
    ]

    with tile.TileContext(nc) as tc, ExitStack() as ctx:
        wpool = ctx.enter_context(tc.tile_pool(name="wpool", bufs=1))
        gpool = ctx.enter_context(tc.tile_pool(name="gpool", bufs=1))
        ampool = ctx.enter_context(tc.tile_pool(name="ampool", bufs=1))
        ptgpool = ctx.enter_context(tc.tile_pool(name="ptgpool", bufs=1))
        hpool = ctx.enter_context(tc.tile_pool(name="hpool", bufs=1))
        tmppool = ctx.enter_context(tc.tile_pool(name="tmppool", bufs=5))
        osbpool = ctx.enter_context(tc.tile_pool(name="osbpool", bufs=2))
        psA = ctx.enter_context(tc.tile_pool(name="psA", bufs=2, space="PSUM"))
        psW = ctx.enter_context(tc.tile_pool(name="psW", bufs=2, space="PSUM"))

        # --- constants (DMAs for these are emitted after the adjacency /
        # x / gather-weight loads, which gate the first matmuls) ---
        wmm_sb = wpool.tile([128, 12, 128], bf16, name="wmm_sb")
        bias_sb = wpool.tile([128, 16], f32, name="bias_sb")
        ident = wpool.tile([128, 128], bf16, name="ident")
        make_identity(nc, ident[:])


        def c_wblk(l, tg, j):
            return wmm_sb[:, (l * NG + tg) * 2 + j, :]

        def d_wblk(l, tg):
            return wmm_sb[:, 8 + l * NG + tg, :]

        # bias columns per layer l: 0: bc(tg0,j0) for the ACT term,
        # 1..3: negated bc for the three STT relu terms, 4: their sum (Bstar),
        # 5..6: d-path biases. Layer stride 7.
        def cb_act(l):
            return bias_sb[:, 7 * l:7 * l + 1]

        def cb_neg(l, k):
            return bias_sb[:, 7 * l + 1 + k:7 * l + 2 + k]

        def cb_star(l):
            return bias_sb[:, 7 * l + 4:7 * l + 5]

        def d_bias(l, tg):
            return bias_sb[:, 7 * l + 5 + tg:7 * l + 6 + tg]

        # --- inputs: x into g c-block 0; adjacency + gather-weight tiles ---
        g = []
        for mt in range(KT):
            gt = gpool.tile([128, 3, T, N_MIX, DIM], bf16, name=f"g{mt}", tag=f"g{mt}")
            g.append(gt)

        # virtual-node rows (node N = partition 116 of the last node tile,
        # plus zeroed pad rows) for c-blocks 1..2: written once here; the
        # transpose drains below never touch partitions >= 116 of g[3]
        nc.sync.dma_start(out=g[KT - 1][116:128, 1:3, :, :, :], in_=hc_d[:])

        adj_sb, wg_sb = [], []
        for kt in range(KT):
            a = ampool.tile([128, NG, NP], bf16, name=f"adj{kt}", tag=f"adj{kt}")
            nc.sync.dma_start(out=a[:], in_=adj_d[kt])
            adj_sb.append(a)
            nc.sync.dma_start(
                out=g[kt][:, 0, :, :, :],
                in_=xn_d[kt * 128:(kt + 1) * 128, :, :, :],
            )
        for kt in range(KT):
            w = ampool.tile([128, NG, NP], bf16, name=f"wg{kt}", tag=f"wg{kt}")
            nc.sync.dma_start(out=w[:], in_=wg_d[kt])
            wg_sb.append(w)
        nc.sync.dma_start(out=wmm_sb[:], in_=wmm_d[:])
        nc.sync.dma_start(out=bias_sb[:], in_=bias_d[:])

        # P^T staging: [128=(j,d), tg, t, m] bf16
        ptg = ptgpool.tile([128, NG, T, NP], bf16, name="ptg", tag="ptg")
        hb = hpool.tile([128, T, NP], bf16, name="hb", tag="hb")

        def tmp_tile():
            return tmppool.tile([128, 2, NP], f32, name="tmp", tag="tmp")

        # alternating engine pickers for psum-reading ops (gpsimd can't)
        ps_rr = [0]

        def ps_copy_rr(out, in_):
            k = ps_rr[0] % 2
            ps_rr[0] += 1
            if k == 0:
                nc.vector.tensor_copy(out, in_)
            else:
                nc.scalar.copy(out, in_)

        # ---------------- gather machinery ----------------
        # units are generators yielding after each kt pair of MMs so the
        # gather stream can micro-interleave with weight matmuls on the PE
        # queue; psum drains are deferred and flushed at slot starts so they
        # sit at the front of the DVE/Scalar queues.
        fd = {"gen": None, "queue": [], "drains": [], "osb": {}}

        def unit_gen(bp, t):
            if bp not in fd["osb"]:
                fd["osb"][bp] = osbpool.tile(
                    [128, NG, NP, T], f32, name=f"osb{bp}", tag="osb")
            pG = psA.tile([128, NG, NP], f32, name="psg", tag="ps")
            for kt in range(KT):
                for tg in range(NG):
                    nc.tensor.matmul(
                        pG[:, tg, :], g[kt][:, bp, t, :, :], wg_sb[kt][:, tg, :],
                        start=(kt == 0), stop=(kt == KT - 1),
                    )
                if kt < KT - 1:
                    yield
            fd["drains"].append((bp, t, pG))

        def take(n):
            for _ in range(n):
                if fd["gen"] is None:
                    if not fd["queue"]:
                        return
                    fd["gen"] = unit_gen(*fd["queue"].pop(0))
                try:
                    next(fd["gen"])
                except StopIteration:
                    fd["gen"] = None

        def flush_drains():
            for bp, t, pG in fd["drains"]:
                osb = fd["osb"][bp]
                ps_copy_rr(osb[:, :, :, t], pG[:])
                if t == T - 1:
                    for tg in range(NG):
                        for i in range(N_MIX):
                            c0 = tg * C + bp * DIM
                            nc.sync.dma_start(
                                out=out_d[i][c0:c0 + DIM, :, :],
                                in_=osb[i * DIM:(i + 1) * DIM, tg, 0:N, :],
                            )
            fd["drains"].clear()

        # ---------------- layer emission ----------------
        def emit_wpair(l, p):
            t2 = slice(2 * p, 2 * p + 2)
            # weight matmuls: preacts WITHOUT bias (bias folded into the
            # elementwise chain below via relu(x+b) = max(x,-b)+b)
            pWs = {}
            for tg in range(NG):
                for blk in range(3):
                    pW = psW.tile([128, 2, NP], f32, name="psw", tag="pw")
                    lhsT = c_wblk(l, tg, blk) if blk < 2 else d_wblk(l, tg)
                    for ti in range(2):
                        nc.tensor.matmul(
                            pW[:, ti, :], lhsT, ptg[:, tg, 2 * p + ti, :],
                            start=True, stop=True,
                        )
                    pWs[(tg, blk)] = pW
                    take(1)
            # chain:
            #  s1 = relu(pW_c00 + bc00)                      [scalar ACT]
            #  s2 = max(pW_c01, -bc01) + s1                  [DVE STT]
            #  s3 = max(pW_c10, -bc10) + s2                  [DVE STT]
            #  s4 = max(pW_c11, -bc11) + s3                  [DVE STT]
            #  t0 = tanh(pW_d0 + bd0), t1 = tanh(pW_d1 + bd1) [scalar ACT]
            #  s5 = s4 + t0                                  [gpsimd TT]
            #  hb = (s5 + Bstar) + t1   (Bstar = bc01+bc10+bc11) [DVE STT]
            s1 = tmp_tile()
            nc.scalar.activation(s1[:], pWs[(0, 0)][:], AF.Relu, bias=cb_act(l))
            s2 = tmp_tile()
            nc.vector.scalar_tensor_tensor(
                s2[:], pWs[(0, 1)][:], cb_neg(l, 0), s1[:],
                op0=ALU.max, op1=ALU.add)
            s3 = tmp_tile()
            nc.vector.scalar_tensor_tensor(
                s3[:], pWs[(1, 0)][:], cb_neg(l, 1), s2[:],
                op0=ALU.max, op1=ALU.add)
            s4 = tmp_tile()
            nc.vector.scalar_tensor_tensor(
                s4[:], pWs[(1, 1)][:], cb_neg(l, 2), s3[:],
                op0=ALU.max, op1=ALU.add)
            t0 = tmp_tile()
            nc.scalar.activation(t0[:], pWs[(0, 2)][:], AF.Tanh, bias=d_bias(l, 0))
            t1 = tmp_tile()
            nc.scalar.activation(t1[:], pWs[(1, 2)][:], AF.Tanh, bias=d_bias(l, 1))
            # tsum runs parallel to the relu chain; hb comes one STT after s4
            tsum = tmp_tile()
            nc.gpsimd.tensor_add(tsum[:], t0[:], t1[:])
            nc.vector.scalar_tensor_tensor(
                hb[:, t2, :], s4[:], cb_star(l), tsum[:],
                op0=ALU.add, op1=ALU.add)

        def emit_layer(l):
            def transpose_half(th):
                for mt in range(KT):
                    pT = psW.tile([128, 6, 128], bf16, name="pstr", tag="pw")
                    for tt in range(6):
                        t = th * 6 + tt
                        nc.tensor.transpose(
                            pT[:, tt, :],
                            hb[:, t, mt * 128:(mt + 1) * 128],
                            ident[:],
                        )
                    dst = g[mt][:, l + 1, th * 6:(th + 1) * 6, :, :]
                    srcT = pT[:].rearrange("p t (i d) -> p t i d", i=N_MIX)
                    if mt == KT - 1:
                        # keep the virtual-node + pad rows (>=116) intact
                        ps_copy_rr(dst[0:96], srcT[0:96])
                        ps_copy_rr(dst[96:116], srcT[96:116])
                    else:
                        ps_copy_rr(dst, srcT)

            for p in range(6):
                flush_drains()
                pP = []
                for t in (2 * p, 2 * p + 1):
                    ps = psA.tile([128, NG, NP], f32, name="psadj", tag="ps")
                    for kt in range(KT):
                        for tg in range(NG):
                            nc.tensor.matmul(
                                ps[:, tg, :], g[kt][:, l, t, :, :],
                                adj_sb[kt][:, tg, :],
                                start=(kt == 0), stop=(kt == KT - 1),
                            )
                    pP.append(ps)
                for ti, t in enumerate((2 * p, 2 * p + 1)):
                    ps_copy_rr(ptg[:, :, t, :], pP[ti][:])
                if p == 5:
                    # first t-half of hb (pairs 0-2) is final: transpose early
                    transpose_half(0)
                    if l + 1 <= 2:
                        fd["queue"].extend((l + 1, t) for t in range(6))
                take(4)
                if p > 0:
                    emit_wpair(l, p - 1)
            emit_wpair(l, 5)
            flush_drains()
            take(8)
            transpose_half(1)
            if l + 1 <= 2:
                fd["queue"].extend((l + 1, t) for t in range(6, T))

        # ---------------- schedule ----------------
        fd["queue"] = [(0, t) for t in range(T)]
        emit_layer(0)
        emit_layer(1)
        while fd["queue"] or fd["gen"] is not None:
            take(4)
            flush_drains()
        flush_drains()

    _dedupe_ldweights(nc)
    nc.compile()
    _BUILT["nc"] = nc
    return nc


def _host_prep(x0, x1, graphs, neighbors, neighbors_weight, a_weight, B_weight,
               a_bias, B_bias):
    """Fuse weights, densify gather, build per-core input maps."""
    import concourse.mybir as mybir
    bf = mybir.dt.np(mybir.dt.bfloat16)
    f = np.float32
    x0 = np.asarray(x0, f)
    x1 = np.asarray(x1, f)
    graphs = np.asarray(graphs, f)
    neighbors = np.asarray(neighbors).astype(np.int64)
    neighbors_weight = np.asarray(neighbors_weight, f)
    a_weight = np.asarray(a_weight, f)
    B_weight = np.asarray(B_weight, f)
    a_bias = np.asarray(a_bias, f)
    B_bias = np.asarray(B_bias, f)

    # fused relation weights: wc/wd [R, L, D, D], bc/bd [R, L, D]
    wc = np.sum(a_weight[0] * B_weight, axis=1)
    wd = np.sum(a_weight[1] * B_weight, axis=1)
    bc = np.sum(a_bias[0] * B_bias, axis=1)
    bd = np.sum(a_bias[1] * B_bias, axis=1)
    # wmm blob: 12 blocks of [128=(j,d), 128=(i,d')].
    # c block (l,tg,j): rows j*64.. hold [wc(tg,0,j) | wc(tg,1,j)], rest zero.
    # d block (l,tg): rows 0:64 = [-wd01 | +wd10], rows 64:128 = [+wd01 | -wd10]
    wmm = np.zeros((128, 12, 128), f)
    for l in range(N_LAYERS):
        for tg in range(NG):
            for j in range(N_MIX):
                blk = (l * NG + tg) * 2 + j
                r0 = j * 64
                wmm[r0:r0 + 64, blk, 0:64] = wc[_rel(tg, 0, j), l]
                wmm[r0:r0 + 64, blk, 64:128] = wc[_rel(tg, 1, j), l]
            blk = 8 + l * NG + tg
            wd01, wd10 = wd[_rel(tg, 0, 1), l], wd[_rel(tg, 1, 0), l]
            wmm[0:64, blk, 0:64] = -wd01
            wmm[0:64, blk, 64:128] = wd10
            wmm[64:128, blk, 0:64] = wd01
            wmm[64:128, blk, 64:128] = -wd10

    bias = np.zeros((128, 16), f)
    hc = np.zeros((12, N_LAYERS, T, N_MIX, DIM), f)

    def bc_col(l, tg, j):
        v = np.zeros(128, f)
        v[0:64] = bc[_rel(tg, 0, j), l]
        v[64:128] = bc[_rel(tg, 1, j), l]
        return v

    for l in range(N_LAYERS):
        # chain term order: ACT=(tg0,j0); STT: (tg0,j1), (tg1,j0), (tg1,j1)
        bias[:, 7 * l] = bc_col(l, 0, 0)
        stt = [bc_col(l, 0, 1), bc_col(l, 1, 0), bc_col(l, 1, 1)]
        for k in range(3):
            bias[:, 7 * l + 1 + k] = -stt[k]
        bias[:, 7 * l + 4] = stt[0] + stt[1] + stt[2]
        for tg in range(NG):
            col = 7 * l + 5 + tg
            bias[0:64, col] = bd[_rel(tg, 0, 1), l]
            bias[64:128, col] = bd[_rel(tg, 1, 0), l]
        for i in range(N_MIX):
            acc = np.zeros(DIM, f)
            for tg in range(NG):
                acc += np.tanh(bd[_rel(tg, i, i), l])
            hc[0, l, :, i, :] = acc

    # adjacency / gather weights with the virtual hconst node at row N:
    # row N carries per-column sums so the constant propagates additively.
    adjp = np.zeros((NG, NP, NP), f)
    adjp[:, :N, :N] = graphs
    adjp[:, N, :N] = graphs.sum(axis=1)
    wgp = np.zeros((NG, NP, NP), f)
    for tg in range(NG):
        np.add.at(
            wgp[tg],
            (neighbors[tg].reshape(-1),
             np.repeat(np.arange(N), K)),
            neighbors_weight[tg].reshape(-1),
        )
    wgp[:, N, :N] = wgp[:, :N, :N].sum(axis=1)
    adjb = np.ascontiguousarray(
        adjp.reshape(NG, KT, 128, NP).transpose(1, 2, 0, 3)).astype(bf)
    wgb = np.ascontiguousarray(
        wgp.reshape(NG, KT, 128, NP).transpose(1, 2, 0, 3)).astype(bf)
    wmmb = wmm.astype(bf)
    hcb = hc.astype(bf)

    in_maps = []
    for b in range(NCORES):
        xn = np.zeros((NP, T, N_MIX, DIM), f)
        xn[:N, :, 0, :] = np.transpose(x0[b], (1, 2, 0))  # [D,N,T] -> [N,T,D]
        xn[:N, :, 1, :] = np.transpose(x1[b], (1, 2, 0))
        in_maps.append({
            "xn": xn.astype(bf), "adjb": adjb, "wgb": wgb, "wmm": wmmb,
            "bias": bias, "hc": hcb,
        })
    return in_maps


def kernel(x0, x1, graphs, neighbors, neighbors_weight, a_weight, B_weight,
           a_bias, B_bias):
    from concourse.bass_utils import run_bass_kernel_spmd

    nc = _build()
    in_maps = _host_prep(x0, x1, graphs, neighbors, neighbors_weight,
                         a_weight, B_weight, a_bias, B_bias)
    trace = bool(int(os.environ.get("KERNEL_TRACE", "0")))
    res = run_bass_kernel_spmd(nc, in_maps, list(range(NCORES)), trace=trace)
    kernel.last_result = res

    out0 = np.stack([res.results[b]["out0"] for b in range(NCORES)])  # [B, 384, 500, 12]
    out1 = np.stack([res.results[b]["out1"] for b in range(NCORES)])
    return out0, out1


kernel.last_result = None
